# revision 1
# baseline (speedup 1.0000x reference)
"""Longformer (dense softmax + rel-pos bias) attention on 8 TRN2 cores.

Sharding: one head per NeuronCore (H=8). Per core:
  qT,kT = Wqk_h @ xT  (packed 128-row lhsT),  v = x @ Wv_h.T
  s[j,i] = kT.T q /8 + bias(i-j)   (scores held transposed: j on partitions)
  a = exp(s)  (no max-subtraction needed: |s| bounded ~<40 for these inputs)
  out[i,d] = sum_j a[j,i] v[j,d] / sum_j a[j,i]   (ones column appended to v)
The rel-pos bias is Toeplitz: only 8 distinct 128x512 tiles per head touch
the non-saturated band; they are host-precomputed. Saturated regions use a
per-head constant folded into the exp() activation bias.
"""

import numpy as np
import sys

sys.path.insert(0, "/opt/trn_rl_repo")

T = 4096
D = 512
H = 8
HD = 64
WIN = 256
NCORES = 8

_cache = {}


def _build():
    from concourse import bacc, bass, tile
    import concourse.mybir as mybir

    f32 = mybir.dt.float32
    f32r = mybir.dt.float32r
    bf16 = mybir.dt.bfloat16
    ADD = mybir.AluOpType.add
    EXP = mybir.ActivationFunctionType.Exp

    nc = bacc.Bacc("TRN2", target_bir_lowering=False, debug=False, num_devices=NCORES)

    xT_d = nc.dram_tensor("xT", (D, T), bf16, kind="ExternalInput")
    wqk_d = nc.dram_tensor("wqk", (128, 512), bf16, kind="ExternalInput")
    wv_d = nc.dram_tensor("wv", (128, 256), bf16, kind="ExternalInput")
    bt_d = nc.dram_tensor("btiles", (128, 8, 512), f32, kind="ExternalInput")
    fc_d = nc.dram_tensor("fconst", (128, 2), f32, kind="ExternalInput")
    out_d = nc.dram_tensor("out", (T, HD), f32, kind="ExternalOutput")

    with tile.TileContext(nc) as tc:
        with (
            tc.tile_pool(name="const", bufs=1) as cpool,
            tc.tile_pool(name="xt", bufs=4) as xpool,
            tc.tile_pool(name="big", bufs=1) as bpool,
        ):
            wqk_sb = cpool.tile([128, 512], bf16, tag="wqk")
            wv_sb = cpool.tile([128, 256], bf16, tag="wv")
            bt_sb = cpool.tile([128, 8, 512], f32, tag="bt")
            fc_sb = cpool.tile([128, 2], f32, tag="fc")
            q_sb = bpool.tile([64, T], bf16, tag="q")
            k_sb = bpool.tile([64, T], bf16, tag="k")
            vh_sb = bpool.tile([128, 32, 65], bf16, tag="vh")

            nc.sync.dma_start(out=wqk_sb[:], in_=wqk_d[:])
            nc.sync.dma_start(out=wv_sb[:], in_=wv_d[:])
            nc.sync.dma_start(out=fc_sb[:], in_=fc_d[:])
            nc.scalar.dma_start(out=bt_sb[:], in_=bt_d[:])

            xt = []
            for kc in range(4):
                t_ = xpool.tile([128, T], bf16, tag="xt")
                xt.append(t_)
                for cb in range(4):
                    eng = nc.sync if (kc * 4 + cb) % 2 == 0 else nc.scalar
                    eng.dma_start(
                        out=t_[:, cb * 1024 : (cb + 1) * 1024],
                        in_=xT_d[kc * 128 : (kc + 1) * 128, cb * 1024 : (cb + 1) * 1024],
                    )

            nc.vector.memset(vh_sb[:, :, 64], 1.0)

            # ---- projections ----
            with tc.tile_pool(name="ppsum", bufs=2, space="PSUM") as ppool:
                for jm in range(8):
                    pq = ppool.tile([64, 512], f32, tag="pq")
                    pk = ppool.tile([64, 512], f32, tag="pk")
                    for kc in range(4):
                        nc.tensor.matmul(
                            pq[:],
                            wqk_sb[:, kc * 128 : kc * 128 + 64],
                            xt[kc][:, jm * 512 : (jm + 1) * 512],
                            start=(kc == 0),
                            stop=(kc == 3),
                        )
                    for kc in range(4):
                        nc.tensor.matmul(
                            pk[:],
                            wqk_sb[:, kc * 128 + 64 : (kc + 1) * 128],
                            xt[kc][:, jm * 512 : (jm + 1) * 512],
                            start=(kc == 0),
                            stop=(kc == 3),
                        )
                    nc.vector.tensor_copy(q_sb[:, jm * 512 : (jm + 1) * 512], pq[:])
                    nc.vector.tensor_copy(k_sb[:, jm * 512 : (jm + 1) * 512], pk[:])
                for jt in range(32):
                    pv = ppool.tile([128, 64], f32, tag="pv")
                    for kc in range(4):
                        nc.tensor.matmul(
                            pv[:],
                            xt[kc][:, jt * 128 : (jt + 1) * 128],
                            wv_sb[:, kc * 64 : (kc + 1) * 64],
                            start=(kc == 0),
                            stop=(kc == 3),
                        )
                    nc.vector.tensor_copy(vh_sb[:, jt, 0:64], pv[:])

            # ---- attention ----
            with (
                tc.tile_pool(name="spsum", bufs=3, space="PSUM") as spool,
                tc.tile_pool(name="opsum", bufs=1, space="PSUM") as opool,
                tc.tile_pool(name="expp", bufs=4) as epool,
                tc.tile_pool(name="outp", bufs=4) as outpool,
            ):
                for im in range(8):
                    po = [
                        opool.tile([128, 65], f32, tag=f"po{ic}", name=f"po{ic}")
                        for ic in range(4)
                    ]
                    for jt in range(32):
                        s = spool.tile([128, 512], f32, tag="s")
                        nc.tensor.matmul(
                            s[:],
                            k_sb[:, jt * 128 : (jt + 1) * 128],
                            q_sb[:, im * 512 : (im + 1) * 512],
                            start=True,
                            stop=True,
                        )
                        delta = im * 512 - jt * 128
                        if delta >= 384:
                            bias_ap = fc_sb[:, 0:1]
                        elif delta <= -768:
                            bias_ap = fc_sb[:, 1:2]
                        else:
                            ti = (delta + 640) // 128
                            nc.vector.tensor_tensor(s[:], s[:], bt_sb[:, ti, :], op=ADD)
                            bias_ap = 0.0
                        e = epool.tile([128, 512], bf16, tag="e")
                        nc.scalar.activation(e[:], s[:], EXP, bias=bias_ap, scale=0.125)
                        for ic in range(4):
                            nc.tensor.matmul(
                                po[ic][:],
                                e[:, ic * 128 : (ic + 1) * 128],
                                vh_sb[:, jt, :],
                                start=(jt == 0),
                                stop=(jt == 31),
                                skip_group_check=True,
                            )
                    for ic in range(4):
                        rec = outpool.tile([128, 1], f32, tag="rec")
                        nc.vector.reciprocal(rec[:], po[ic][:, 64:65])
                        ob = outpool.tile([128, 64], f32, tag="ob")
                        nc.vector.tensor_scalar_mul(ob[:], po[ic][:, 0:64], rec[:])
                        r0 = (im * 4 + ic) * 128
                        nc.sync.dma_start(out=out_d[r0 : r0 + 128, :], in_=ob[:])

    nc.compile()
    return nc


def _prep_inputs(x, Wq, Wk, Wv, rel_pos_bias):
    import ml_dtypes

    bf = ml_dtypes.bfloat16
    xT = np.ascontiguousarray(x[0].T.astype(np.float32)).astype(bf)  # (D, T)
    in_maps = []
    for h in range(H):
        WqT = Wq[h * HD : (h + 1) * HD, :].T.astype(np.float32)  # (D, 64)
        WkT = Wk[h * HD : (h + 1) * HD, :].T.astype(np.float32)
        WvT = Wv[h * HD : (h + 1) * HD, :].T.astype(np.float32)
        wqkT = np.concatenate([WqT, WkT], axis=1)  # (512, 128)
        wqk = wqkT.reshape(4, 128, 128).transpose(1, 0, 2).reshape(128, 512)
        wv = WvT.reshape(4, 128, 64).transpose(1, 0, 2).reshape(128, 256)

        tab = rel_pos_bias[0, h].astype(np.float32)  # (511,)
        dgrid = np.arange(-640, -640 + 8 * 128, 128)[:, None, None] + (
            np.arange(512)[None, None, :] - np.arange(128)[None, :, None]
        )  # (8, 128, 512) values of d = i - j
        q = tab[np.clip(dgrid, -(WIN - 1), WIN - 1) + (WIN - 1)]
        btiles = np.ascontiguousarray((8.0 * q).transpose(1, 0, 2).astype(np.float32))
        fconst = np.empty((128, 2), np.float32)
        fconst[:, 0] = tab[510]
        fconst[:, 1] = tab[0]
        in_maps.append(
            {
                "xT": xT,
                "wqk": np.ascontiguousarray(wqk).astype(bf),
                "wv": np.ascontiguousarray(wv).astype(bf),
                "btiles": btiles,
                "fconst": fconst,
            }
        )
    return in_maps


def kernel(x, Wq, Wk, Wv, rel_pos_bias, _trace=False):
    from concourse import bass_utils

    if "nc" not in _cache:
        _cache["nc"] = _build()
    nc = _cache["nc"]
    in_maps = _prep_inputs(x, Wq, Wk, Wv, rel_pos_bias)
    res = bass_utils.run_bass_kernel_spmd(
        nc, in_maps, core_ids=list(range(NCORES)), trace=_trace
    )
    _cache["last_result"] = res
    out = np.empty((1, T, D), np.float32)
    for h in range(H):
        out[0, :, h * HD : (h + 1) * HD] = res.results[h]["out"]
    return out



# revision 2
# speedup vs baseline: 1.0182x; 1.0182x over previous
"""Longformer (dense softmax + rel-pos bias) attention on 8 TRN2 cores.

Sharding: one head per NeuronCore (H=8). Per core:
  qT,kT = Wqk_h @ xT  (packed 128-row lhsT),  v = x @ Wv_h.T
  s[j,i] = kT.T q /8 + bias(i-j)   (scores held transposed: j on partitions)
  a = exp(s)  (no max-subtraction needed: |s| bounded ~<40 for these inputs)
  outT[d,i] = sum_j v[j,d] a[j,i];  row 64 of v-block is ones -> denominator
  out[i,d] = outT[d,i] / outT[64,i]  (via PE transpose, then per-row scale)

v2 layout notes vs v1:
  - AV matmuls keep V as the stationary operand (one LDWEIGHTS per j-tile)
    and stream the exp tile as a fat 512-wide moving operand, so the PE
    array stays busy and the HAM clock gate holds 2.4 GHz.
  - exp() runs 1024 elements per ACTIVATE (two score tiles share a 2-bank
    PSUM tile) to amortize the ~352-cycle per-instruction ACT overhead.
  - The rel-pos bias is Toeplitz: in-band tiles add a host-precomputed
    f32 tile in PSUM before exp; saturated regions instead use V copies
    pre-scaled by exp(bias_const), so they cost nothing per tile.
"""

import numpy as np
import sys

sys.path.insert(0, "/opt/trn_rl_repo")

T = 4096
D = 512
H = 8
HD = 64
WIN = 256
NCORES = 8

_cache = {}


def _build():
    from concourse import bacc, bass, tile
    import concourse.mybir as mybir

    f32 = mybir.dt.float32
    bf16 = mybir.dt.bfloat16
    ADD = mybir.AluOpType.add
    EXP = mybir.ActivationFunctionType.Exp

    nc = bacc.Bacc("TRN2", target_bir_lowering=False, debug=False, num_devices=NCORES)

    xT_d = nc.dram_tensor("xT", (D, T), bf16, kind="ExternalInput")
    wqk_d = nc.dram_tensor("wqk", (128, 512), bf16, kind="ExternalInput")
    wv_d = nc.dram_tensor("wv", (128, 256), bf16, kind="ExternalInput")
    bt_d = nc.dram_tensor("btiles", (128, 8, 512), f32, kind="ExternalInput")
    fc_d = nc.dram_tensor("fconst", (128, 2), f32, kind="ExternalInput")
    id_d = nc.dram_tensor("ident", (65, 65), f32, kind="ExternalInput")
    out_d = nc.dram_tensor("out", (T, HD), f32, kind="ExternalOutput")

    with tile.TileContext(nc) as tc:
        with (
            tc.tile_pool(name="const", bufs=1) as cpool,
            tc.tile_pool(name="xt", bufs=4) as xpool,
            tc.tile_pool(name="big", bufs=1) as bpool,
        ):
            wqk_sb = cpool.tile([128, 512], bf16, tag="wqk")
            wv_sb = cpool.tile([128, 256], bf16, tag="wv")
            bt_sb = cpool.tile([128, 8, 512], f32, tag="bt")
            fc_sb = cpool.tile([128, 2], f32, tag="fc")
            id_sb = cpool.tile([65, 65], f32, tag="ident")
            q_sb = bpool.tile([64, T], bf16, tag="q")
            k_sb = bpool.tile([64, T], bf16, tag="k")
            vh_sb = bpool.tile([128, 32, 65], bf16, tag="vh")
            vhh_sb = bpool.tile([128, 32, 65], bf16, tag="vhh")
            vhl_sb = bpool.tile([128, 32, 65], bf16, tag="vhl")

            nc.sync.dma_start(out=wqk_sb[:], in_=wqk_d[:])
            nc.sync.dma_start(out=wv_sb[:], in_=wv_d[:])
            nc.sync.dma_start(out=fc_sb[:], in_=fc_d[:])
            nc.sync.dma_start(out=id_sb[:], in_=id_d[:])
            nc.scalar.dma_start(out=bt_sb[:], in_=bt_d[:])

            xt = []
            for kc in range(4):
                t_ = xpool.tile([128, T], bf16, tag="xt")
                xt.append(t_)
                for cb in range(4):
                    eng = nc.sync if (kc * 4 + cb) % 2 == 0 else nc.scalar
                    eng.dma_start(
                        out=t_[:, cb * 1024 : (cb + 1) * 1024],
                        in_=xT_d[kc * 128 : (kc + 1) * 128, cb * 1024 : (cb + 1) * 1024],
                    )

            nc.vector.memset(vh_sb[:, :, 64], 1.0)

            # ---- projections ----
            with tc.tile_pool(name="ppsum", bufs=2, space="PSUM") as ppool:
                for jm in range(8):
                    pq = ppool.tile([64, 512], f32, tag="pq")
                    pk = ppool.tile([64, 512], f32, tag="pk")
                    for kc in range(4):
                        nc.tensor.matmul(
                            pq[:],
                            wqk_sb[:, kc * 128 : kc * 128 + 64],
                            xt[kc][:, jm * 512 : (jm + 1) * 512],
                            start=(kc == 0),
                            stop=(kc == 3),
                        )
                    for kc in range(4):
                        nc.tensor.matmul(
                            pk[:],
                            wqk_sb[:, kc * 128 + 64 : (kc + 1) * 128],
                            xt[kc][:, jm * 512 : (jm + 1) * 512],
                            start=(kc == 0),
                            stop=(kc == 3),
                        )
                    nc.vector.tensor_copy(q_sb[:, jm * 512 : (jm + 1) * 512], pq[:])
                    nc.vector.tensor_copy(k_sb[:, jm * 512 : (jm + 1) * 512], pk[:])
                for jt in range(32):
                    pv = ppool.tile([128, 64], f32, tag="pv")
                    for kc in range(4):
                        nc.tensor.matmul(
                            pv[:],
                            xt[kc][:, jt * 128 : (jt + 1) * 128],
                            wv_sb[:, kc * 64 : (kc + 1) * 64],
                            start=(kc == 0),
                            stop=(kc == 3),
                        )
                    nc.scalar.copy(vh_sb[:, jt, 0:64], pv[:])
                # saturated-bias V copies: exp(b_hi)*v and exp(b_lo)*v
                nc.vector.tensor_scalar_mul(vhh_sb[:], vh_sb[:], fc_sb[:, 0:1])
                nc.vector.tensor_scalar_mul(vhl_sb[:], vh_sb[:], fc_sb[:, 1:2])

            # ---- attention ----
            with (
                tc.tile_pool(name="spsum", bufs=2, space="PSUM") as spool,
                tc.tile_pool(name="opsum", bufs=2, space="PSUM") as opool,
                tc.tile_pool(name="tpsum", bufs=2, space="PSUM") as tpool,
                tc.tile_pool(name="expp", bufs=3) as epool,
                tc.tile_pool(name="outp", bufs=4) as outpool,
            ):
                for im in range(8):
                    poT = opool.tile([65, 512], f32, tag="poT", name=f"poT{im}")
                    for jj in range(16):
                        s2 = spool.tile([128, 1024], f32, tag="s2")
                        for u in range(2):
                            jt = 2 * jj + u
                            nc.tensor.matmul(
                                s2[:, u * 512 : (u + 1) * 512],
                                k_sb[:, jt * 128 : (jt + 1) * 128],
                                q_sb[:, im * 512 : (im + 1) * 512],
                                start=True,
                                stop=True,
                                skip_group_check=True,
                            )
                        for u in range(2):
                            jt = 2 * jj + u
                            delta = im * 512 - jt * 128
                            if -768 < delta < 384:
                                ti = (delta + 640) // 128
                                nc.vector.tensor_tensor(
                                    s2[:, u * 512 : (u + 1) * 512],
                                    s2[:, u * 512 : (u + 1) * 512],
                                    bt_sb[:, ti, :],
                                    op=ADD,
                                )
                        e2 = epool.tile([128, 1024], bf16, tag="e2")
                        nc.scalar.activation(e2[:], s2[:], EXP, bias=0.0, scale=0.125)
                        for u in range(2):
                            jt = 2 * jj + u
                            delta = im * 512 - jt * 128
                            if delta >= 384:
                                stat = vhh_sb
                            elif delta <= -768:
                                stat = vhl_sb
                            else:
                                stat = vh_sb
                            nc.tensor.matmul(
                                poT[:],
                                stat[:, jt, :],
                                e2[:, u * 512 : (u + 1) * 512],
                                start=(jt == 0),
                                stop=(jt == 31),
                                skip_group_check=True,
                            )
                    poc = outpool.tile([65, 512], f32, tag="poc")
                    nc.vector.tensor_copy(poc[:], poT[:])
                    ot = tpool.tile([128, 4, 65], f32, tag="ot", name=f"ot{im}")
                    for ic in range(4):
                        nc.tensor.transpose(
                            ot[:, ic, :], poc[:, ic * 128 : (ic + 1) * 128], id_sb[:]
                        )
                    for ic in range(4):
                        rec = outpool.tile([128, 1], f32, tag="rec")
                        nc.vector.reciprocal(rec[:], ot[:, ic, 64:65])
                        ob = outpool.tile([128, 64], f32, tag="ob")
                        nc.vector.tensor_scalar_mul(ob[:], ot[:, ic, 0:64], rec[:])
                        r0 = (im * 4 + ic) * 128
                        nc.sync.dma_start(out=out_d[r0 : r0 + 128, :], in_=ob[:])

    nc.compile()
    return nc


def _prep_inputs(x, Wq, Wk, Wv, rel_pos_bias):
    import ml_dtypes

    bf = ml_dtypes.bfloat16
    xT = np.ascontiguousarray(x[0].T.astype(np.float32)).astype(bf)  # (D, T)
    ident = np.eye(65, dtype=np.float32)
    in_maps = []
    for h in range(H):
        WqT = Wq[h * HD : (h + 1) * HD, :].T.astype(np.float32)  # (D, 64)
        WkT = Wk[h * HD : (h + 1) * HD, :].T.astype(np.float32)
        WvT = Wv[h * HD : (h + 1) * HD, :].T.astype(np.float32)
        wqkT = np.concatenate([WqT, WkT], axis=1)  # (512, 128)
        wqk = wqkT.reshape(4, 128, 128).transpose(1, 0, 2).reshape(128, 512)
        wv = WvT.reshape(4, 128, 64).transpose(1, 0, 2).reshape(128, 256)

        tab = rel_pos_bias[0, h].astype(np.float32)  # (511,)
        dgrid = np.arange(-640, -640 + 8 * 128, 128)[:, None, None] + (
            np.arange(512)[None, None, :] - np.arange(128)[None, :, None]
        )  # (8, 128, 512) values of d = i - j
        q = tab[np.clip(dgrid, -(WIN - 1), WIN - 1) + (WIN - 1)]
        btiles = np.ascontiguousarray((8.0 * q).transpose(1, 0, 2).astype(np.float32))
        fconst = np.empty((128, 2), np.float32)
        fconst[:, 0] = np.exp(tab[510])
        fconst[:, 1] = np.exp(tab[0])
        in_maps.append(
            {
                "xT": xT,
                "wqk": np.ascontiguousarray(wqk).astype(bf),
                "wv": np.ascontiguousarray(wv).astype(bf),
                "btiles": btiles,
                "fconst": fconst,
                "ident": ident,
            }
        )
    return in_maps


def kernel(x, Wq, Wk, Wv, rel_pos_bias, _trace=False):
    from concourse import bass_utils

    if "nc" not in _cache:
        _cache["nc"] = _build()
    nc = _cache["nc"]
    in_maps = _prep_inputs(x, Wq, Wk, Wv, rel_pos_bias)
    res = bass_utils.run_bass_kernel_spmd(
        nc, in_maps, core_ids=list(range(NCORES)), trace=_trace
    )
    _cache["last_result"] = res
    out = np.empty((1, T, D), np.float32)
    for h in range(H):
        out[0, :, h * HD : (h + 1) * HD] = res.results[h]["out"]
    return out
